# revision 4
# baseline (speedup 1.0000x reference)
# Causal self-attention on 8 NeuronCores (Trainium2, Bass/Tile).
#
# Sharding: core c -> batch b = c//4, head-group hg = c%4 (4 of 16 heads).
# Each core computes Q/K/V projections for its heads, causal attention, and
# a partial output projection (its heads' rows of W_out). Host sums the 4
# partials per batch (Megatron-style TP combine); b_out folded in via `bo`
# on hg==0 cores.
#
# v2: streaming + software pipelining. Causality means q-block qs only
# needs K/V from t-blocks <= qs, so the whole kernel is one pass over
# t-blocks. The PE is in-order, and the score+exp stream is ACT-bound
# (~570ns/tile exp vs ~213ns/tile matmul), so projection chains for
# t-block ts+1 and out-projection chains for ts-1 are interleaved as
# fillers between score matmuls — the PE rides through exp waits doing
# useful work. All matmuls N-trim to the causal region.
#
# Engine budget: PE ~113us, ACT ~89us (exp only + store triggers),
# DVE ~70us (psum->sbuf copies, masks, normalize, out-bias),
# Pool ~16us (broadcasts), SP: input DMA, ACT queue: output DMA.
#
# Device layouts (host-prepped, bf16 matmul operands):
#   xT  [1024, 2048]  x[b].T            -> rhs/lhsT chunks over e
#   wq/wk/wv [1024, 256]  W_qkv col-slices (wq pre-scaled by 1/sqrt(D))
#   wo  [256, 1024]   W_out row-slice
# Attention runs in S^T = K Q^T layout ([k, q]); the PV matmul uses
# lhsT = [V | 1] so PSUM row 64 accumulates the softmax denominator.

import os
import numpy as np
import ml_dtypes

import concourse.bass as bass
import concourse.mybir as mybir
import concourse.tile as tile
from concourse import bacc
from concourse.bass_utils import run_bass_kernel_spmd

F32 = mybir.dt.float32
BF16 = mybir.dt.bfloat16
AF = mybir.ActivationFunctionType
OP = mybir.AluOpType

T = 2048
E = 1024
D = 64
NH = 16
H_CORE = 4          # heads per core
PAIRS = 2           # head pairs per core
EC = E // 128       # 8 e-chunks
NT4 = T // 512      # 4 t-tiles of 512
NKT = T // 128      # 16 k-tiles of 128

_cache = {}


def _build(reps=1):
    nc = bacc.Bacc(None, target_bir_lowering=False)
    xT = nc.declare_dram_parameter("xT", [E, T], BF16, isOutput=False)
    wq = nc.declare_dram_parameter("wq", [E, 256], BF16, isOutput=False)
    wk = nc.declare_dram_parameter("wk", [E, 256], BF16, isOutput=False)
    wv = nc.declare_dram_parameter("wv", [E, 256], BF16, isOutput=False)
    wo = nc.declare_dram_parameter("wo", [256, E], BF16, isOutput=False)
    bq = nc.declare_dram_parameter("bq", [128, 2], F32, isOutput=False)
    bk = nc.declare_dram_parameter("bk", [128, 2], F32, isOutput=False)
    bv = nc.declare_dram_parameter("bv", [1, 256], F32, isOutput=False)
    bo = nc.declare_dram_parameter("bo", [1, E], F32, isOutput=False)
    out = nc.declare_dram_parameter("out", [T, E], BF16, isOutput=True)

    xT_r = xT.rearrange("(c p) t -> p c t", p=128)
    wq_r = wq.rearrange("(c p) m -> p c m", p=128)
    wk_r = wk.rearrange("(c p) m -> p c m", p=128)
    wv_r = wv.rearrange("(c p) m -> p c m", p=128)
    wo_r = wo.rearrange("(c p) n -> p c n", p=128)

    import contextlib

    with tile.TileContext(nc) as tc:
        with (
            tc.tile_pool(name="w", bufs=1) as w,
            tc.tile_pool(name="pt", bufs=6) as ptp,
            tc.tile_pool(name="misc", bufs=4) as misc,
            tc.tile_pool(name="ob", bufs=16) as obp,
            tc.For_i(0, reps, 1) if reps > 1 else contextlib.nullcontext(),
        ):
            # ---- static tiles ----
            XT = w.tile([128, EC, T], BF16)
            WQ = w.tile([128, EC, 256], BF16)
            WK = w.tile([128, EC, 256], BF16)
            WV = w.tile([128, EC, 256], BF16)
            WO = w.tile([128, 2, E], BF16)
            BQ = w.tile([128, 2], F32)
            BK = w.tile([128, 2], F32)
            BV1 = w.tile([1, 256], F32)
            BO1 = w.tile([1, E], F32)
            # load order = first-use order; weights on the ACT queue,
            # activations on the SP queue (two HWDGE engines in parallel)
            for c in range(EC):
                nc.sync.dma_start(WQ[:, c, :], wq_r[:, c, :])
                nc.sync.dma_start(XT[:, c, bass.ts(0, 512)],
                                  xT_r[:, c, bass.ts(0, 512)])
            nc.sync.dma_start(WK[:], wk_r[:])
            nc.sync.dma_start(BQ[:], bq[:])
            nc.sync.dma_start(BK[:], bk[:])
            nc.sync.dma_start(BV1[:], bv[:])
            nc.sync.dma_start(BO1[:], bo[:])
            nc.sync.dma_start(WV[:], wv_r[:])
            for ts in range(1, NT4):
                for c in range(EC):
                    nc.sync.dma_start(XT[:, c, bass.ts(ts, 512)],
                                      xT_r[:, c, bass.ts(ts, 512)])
            nc.sync.dma_start(WO[:], wo_r[:])
            BVB = w.tile([128, 256], F32)
            nc.gpsimd.partition_broadcast(BVB[:], BV1[:])
            BOB = w.tile([128, E], F32)
            nc.gpsimd.partition_broadcast(BOB[:], BO1[:])

            # triangular keep-mask [k, j]: keep j >= k
            M01 = w.tile([128, 128], BF16)
            nc.vector.memset(M01[:], 1.0)
            nc.gpsimd.affine_select(
                out=M01[:], in_=M01[:], compare_op=OP.is_ge, fill=0.0,
                base=0, pattern=[[1, 128]], channel_multiplier=-1,
            )

            QT = w.tile([128, PAIRS, T], BF16, tag="QT")
            KT = w.tile([128, PAIRS, T], BF16)
            # V with ones column: [t-part, kt, head, 65]
            VS = w.tile([128, NKT, H_CORE, 65], BF16)
            nc.gpsimd.memset(VS[:, :, :, 64], 1.0)
            # O^T, heads pair-stacked: [d-part, pair, t]
            OT = w.tile([128, PAIRS, T], BF16, tag="OT")

            with (
                tc.tile_pool(name="psP", bufs=2, space="PSUM") as psP,
                tc.tile_pool(name="psS", bufs=2, space="PSUM") as psS,
                tc.tile_pool(name="psO", bufs=2, space="PSUM") as psO,
            ):
                # ---- filler chains (emitted between score matmuls) ----
                def proj_fillers(ts, rings=((psP, "proj"),)):
                    """8 chains: pq/pk per pair + 4 pv t-subtiles."""
                    sl = bass.ts(ts, 512)
                    chains = []
                    tagc = [0]

                    def nextring():
                        r = rings[tagc[0] % len(rings)]
                        tagc[0] += 1
                        return r

                    def qk_chain(p, which):
                        def emit():
                            W_, B_, O_ = ((WQ, BQ, QT) if which == "q"
                                          else (WK, BK, KT))
                            pool_, tag_ = nextring()
                            ps = pool_.tile([128, 512], F32, tag=tag_,
                                            name="p" + which)
                            for e in range(EC):
                                nc.tensor.matmul(
                                    ps[:], W_[:, e, bass.ts(p, 128)],
                                    XT[:, e, sl],
                                    start=(e == 0), stop=(e == EC - 1))
                            nc.vector.tensor_scalar_add(O_[:, p, sl], ps[:],
                                                        B_[:, p:p + 1])
                        return emit

                    def v_chain(tt):
                        def emit():
                            pool_, tag_ = nextring()
                            pv = pool_.tile([128, 512], F32, tag=tag_,
                                            name="pv")
                            for e in range(EC):
                                nc.tensor.matmul(
                                    pv[:, 0:256], XT[:, e, bass.ts(tt, 128)],
                                    WV[:, e, :],
                                    start=(e == 0), stop=(e == EC - 1))
                            nc.vector.tensor_tensor(
                                VS[:, tt, :, 0:64],
                                pv[:, 0:256].rearrange("p (h d) -> p h d",
                                                       h=H_CORE),
                                BVB[:].rearrange("p (h d) -> p h d",
                                                 h=H_CORE),
                                OP.add)
                        return emit

                    for p in range(PAIRS):
                        chains.append(qk_chain(p, "q"))
                        chains.append(qk_chain(p, "k"))
                    for tt in range(4 * ts, 4 * ts + 4):
                        chains.append(v_chain(tt))
                    return chains

                def outproj_fillers(ts, store_list):
                    """8 chains: out-proj tiles; obs appended to store_list."""
                    chains = []

                    def chain(tt, ns):
                        def emit():
                            pu = psP.tile([128, 512], F32, tag="proj",
                                          name="pu")
                            for jc in range(2):
                                nc.tensor.matmul(
                                    pu[:], OT[:, jc, bass.ts(tt, 128)],
                                    WO[:, jc, bass.ts(ns, 512)],
                                    start=(jc == 0), stop=(jc == 1))
                            ob = obp.tile([128, 512], BF16, tag="ob")
                            nc.vector.tensor_tensor(
                                ob[:], pu[:], BOB[:, bass.ts(ns, 512)], OP.add)
                            store_list.append((tt, ns, ob))
                        return emit

                    for tt in range(4 * ts, 4 * ts + 4):
                        for ns in range(2):
                            chains.append(chain(tt, ns))
                    return chains

                def flush_stores(store_list):
                    for tt, ns, ob in store_list:
                        nc.sync.dma_start(
                            out[bass.ts(tt, 128), bass.ts(ns, 512)], ob[:])
                    store_list.clear()

                def attn(qs, fillers, prev_stores, fillers_p1=()):
                    qsl = bass.ts(qs, 512)
                    ktmax = 4 * qs + 4
                    LAG = min(3, ktmax - 1)

                    def pv_mm(pO, p, h, kt, PTs):
                        d = kt - 4 * qs
                        lo = 128 * d if d > 0 else 0
                        nc.tensor.matmul(
                            pO[h][0:65, lo:512],
                            VS[:, kt, 2 * p + h, :],
                            PTs[(kt, h)][:, lo:512],
                            start=(kt == 0), stop=(kt == ktmax - 1))

                    for p in range(PAIRS):
                        fq = list(fillers if p == 0 else fillers_p1)
                        nslots = ktmax
                        emitted = 0
                        slot = 0

                        def maybe_fill():
                            nonlocal emitted, slot
                            slot += 1
                            want = min((slot * len(fq)) // nslots, len(fq))
                            while emitted < want:
                                fq[emitted]()
                                emitted += 1

                        # scores + exp (+ diag mask) with PV matmuls
                        # interleaved LAG tiles behind (PV(kt) only needs
                        # exp(kt), so the PE never waits for the full row)
                        PTs = {}
                        pO = {h: psO.tile([128, 512], F32, tag=f"pO{h}",
                                          name=f"pO{h}") for h in range(2)}
                        for kt in range(ktmax):
                            d = kt - 4 * qs
                            lo = 128 * d if d > 0 else 0
                            for h in range(2):
                                ps = psS.tile([128, 512], F32, tag="pS",
                                              name="ps")
                                nc.tensor.matmul(
                                    ps[:, lo:512],
                                    KT[bass.ts(h, 64), p, bass.ts(kt, 128)],
                                    QT[bass.ts(h, 64), p,
                                       bass.ds(qs * 512 + lo, 512 - lo)],
                                    start=True, stop=True)
                                PT = ptp.tile([128, 512], BF16, tag=f"PT{h}",
                                              name=f"PT{h}", bufs=16)
                                nc.scalar.activation(
                                    PT[:, lo:512], ps[:, lo:512], AF.Exp)
                                if d >= 0:
                                    nc.vector.tensor_tensor(
                                        PT[:, lo:lo + 128], PT[:, lo:lo + 128],
                                        M01[:], OP.mult)
                                PTs[(kt, h)] = PT
                            maybe_fill()
                            if kt >= LAG:
                                for h in range(2):
                                    pv_mm(pO, p, h, kt - LAG, PTs)
                        for kt in range(ktmax - LAG, ktmax):
                            for h in range(2):
                                pv_mm(pO, p, h, kt, PTs)
                        # normalize: O^T = pO[0:64] / den, den = row 64
                        for h in range(2):
                            RL = misc.tile([1, 512], F32, tag="RL")
                            nc.vector.reciprocal(RL[:], pO[h][64:65, :])
                            RLB = misc.tile([64, 512], F32, tag="RLB")
                            nc.gpsimd.partition_broadcast(RLB[:], RL[:])
                            nc.vector.tensor_tensor(
                                OT[bass.ts(h, 64), p, qsl], pO[h][0:64, :],
                                RLB[:], OP.mult)
                        if p == 0 and prev_stores:
                            flush_stores(prev_stores)
                    if prev_stores:
                        flush_stores(prev_stores)

                # ---- streaming schedule ----
                # out-proj(ts) chains are deferred into attn(ts+2) so the
                # filler supply matches each block's exp deficit (attn(3)
                # has the most exp work and no projections left to run).
                st = [[] for _ in range(NT4)]
                for ch in proj_fillers(0, rings=((psP, "proj"), (psS, "pS"))):
                    ch()
                p1f = proj_fillers(1)
                attn(0, p1f[:4], None, fillers_p1=p1f[4:])
                p2f = proj_fillers(2)
                attn(1, p2f[:4], None, fillers_p1=p2f[4:])
                # proj(3): q/k of pair0 must precede attn(3) entirely; the
                # pair1 q/k and all v chains ride inside attn(3)'s pair0 half
                p3 = proj_fillers(3)
                op0 = outproj_fillers(0, st[0])
                attn(2, [p3[0], p3[1]] + op0[:4], st[0], fillers_p1=op0[4:])
                op1 = outproj_fillers(1, st[1])
                op2 = outproj_fillers(2, st[2])
                attn(3, p3[4:] + [p3[2], p3[3]] + op1, st[1],
                     fillers_p1=op2)
                flush_stores(st[2])
                for ch in outproj_fillers(NT4 - 1, st[3]):
                    ch()
                flush_stores(st[3])

    nc.compile()
    return nc


def _in_maps(x, W_qkv, b_qkv, W_out, b_out):
    bf = ml_dtypes.bfloat16
    scale = np.float32(1.0 / np.sqrt(D))
    maps = []
    for c in range(8):
        b, hg = c // 4, c % 4
        qc = slice(hg * 256, hg * 256 + 256)
        m = {
            "xT": np.ascontiguousarray(x[b].T).astype(bf),
            "wq": (W_qkv[:, qc.start:qc.stop] * scale).astype(bf),
            "wk": W_qkv[:, E + qc.start:E + qc.stop].astype(bf),
            "wv": W_qkv[:, 2 * E + qc.start:2 * E + qc.stop].astype(bf),
            "wo": np.ascontiguousarray(W_out[qc, :]).astype(bf),
            "bq": (b_qkv[qc] * scale).astype(np.float32).reshape(2, 128).T.copy(),
            "bk": b_qkv[E + qc.start:E + qc.stop].astype(np.float32).reshape(2, 128).T.copy(),
            "bv": b_qkv[2 * E + qc.start:2 * E + qc.stop].astype(np.float32).reshape(1, 256).copy(),
            "bo": (b_out.astype(np.float32) if hg == 0
                   else np.zeros(E, np.float32)).reshape(1, E).copy(),
        }
        maps.append(m)
    return maps


def kernel(x, W_qkv, b_qkv, W_out, b_out):
    x = np.asarray(x, np.float32)
    W_qkv = np.asarray(W_qkv, np.float32)
    b_qkv = np.asarray(b_qkv, np.float32)
    W_out = np.asarray(W_out, np.float32)
    b_out = np.asarray(b_out, np.float32)
    if "nc" not in _cache:
        _cache["nc"] = _build()
    nc = _cache["nc"]
    maps = _in_maps(x, W_qkv, b_qkv, W_out, b_out)
    res = run_bass_kernel_spmd(nc, maps, list(range(8))).results
    out = np.empty((2, T, E), np.float32)
    for b in range(2):
        acc = res[b * 4]["out"].astype(np.float32)
        for hg in range(1, 4):
            acc = acc + res[b * 4 + hg]["out"]
        out[b] = acc
    return out
